# revision 54
# baseline (speedup 1.0000x reference)
"""Trainium2 Bass kernel for nn_BasicBlock (3-layer GCN block with residual).

Math (per batch item b, per conv):
    out = A @ (x @ W) + bias,  A = normalized adjacency (with self loops)
where A[c, r] = sum over edges r->c of dinv[r]*dinv[c] (dense N x N, shared
across batch and precomputed on host from the edge lists).

Block:
    a1 = relu(A_sp @ (x  @ W1) + b1)
    a2 = relu(A_tm @ (a1 @ W2) + b2)
    o3 =      A_sp @ (a2 @ W3) + b3
    out = relu(o3 + x)

Every layer is the same two matmuls (P=128 partitions):
    W-mm (natural out): h[n,c]  = sum_ci aT[ci,n] * W[ci,c]
        lhsT = aT chunk [ci, 128 n-cols] (14 ldweights), rhs = W, F=256
    A-mm (transposed out): a'T[c,n] = act(sum_m h[m,c] * AT[m,n] + b)
        lhsT = h chunk [m-pair, 128 c] (14 ldweights), rhs = AT streamed
        F=1700 in 512-col PSUM quarters -- ldweights amortized over 4
        matmuls; bias+relu (layers 1-2) or +xT residual+relu (layer 3)
        are applied in the PSUM drain.
Layer 1 uses associativity: A_sp @ (x W1) with h1 = x @ W1 from a
host-transposed fp8 x; layer 3 writes outT [c,n] (host transposes back)
and its bias b3 rides the Act-engine drain bias.  For the final item the
residual is added on the PE via an identity matmul so the unoverlapped
drain is just relu+store.

All matmuls run in fp8 e4m3 with DoubleRow perf mode (two 128-deep k-tiles
contracted per instruction) and fp32 PSUM accumulation. The conv-path signal
is tiny relative to the identity residual, so fp8 error washes out; the
residual itself stays bf16 (x arrives from host as fp8 transposed [c,n] for
layer 1 and bf16 transposed [c,n] for the residual).
Batch (64) is sharded 8 items/core over the 8 cores; A/W/b are replicated.
"""

import sys

if "/opt/trn_rl_repo" not in sys.path:
    sys.path.insert(0, "/opt/trn_rl_repo")

import numpy as np
import ml_dtypes

import concourse.bass as bass
import concourse.bacc as bacc
import concourse.mybir as mybir
import concourse.tile as tile
from concourse.bass_utils import run_bass_kernel_spmd

P = 128
B, N, C = 64, 1700, 256
N_CORES = 8
B_LOCAL = B // N_CORES

F32 = mybir.dt.float32
BF16 = mybir.dt.bfloat16
F8 = mybir.dt.float8e4
RELU = mybir.ActivationFunctionType.Relu
DR = mybir.MatmulPerfMode.DoubleRow
NP_BF16 = ml_dtypes.bfloat16
NP_F8 = ml_dtypes.float8_e4m3


def _quarters(total, step=512):
    return [(q, min(step, total - q)) for q in range(0, total, step)]


def build_program(bl, n, c):
    """Build the Bass/Tile program for `bl` batch items, `n` nodes, `c` chans."""
    kt = -(-(n + 1) // P)  # node chunks, incl >= one zero pad row
    assert kt % 2 == 0, "DoubleRow pairing needs an even k-tile count"
    kp = kt // 2
    npad = kt * P
    npr = -(-n // 16) * 16  # at-tile row pitch: DoubleRow needs 16B-aligned strides
    ct = c // P

    nqv = _quarters(n)  # valid-column quarters (pads are never read)

    nc = bacc.Bacc("TRN2", target_bir_lowering=False, debug=False,
                   enable_asserts=False)

    xt8_d = nc.dram_tensor("xt8", [bl, c, npad], F8, kind="ExternalInput")
    xt_d = nc.dram_tensor("xt", [bl, c, n], BF16, kind="ExternalInput")
    atsp_d = nc.dram_tensor("at_sp", [P, kt, npr], F8, kind="ExternalInput")
    attm_d = nc.dram_tensor("at_tm", [P, kt, npr], F8, kind="ExternalInput")
    w_d = [nc.dram_tensor(f"w{i}", [P, ct, c], F8, kind="ExternalInput")
           for i in (1, 2, 3)]
    b_d = [nc.dram_tensor(f"b{i}", [P, ct], F32, kind="ExternalInput")
           for i in (1, 2, 3)]
    id_d = nc.dram_tensor("ident", [P, P], BF16, kind="ExternalInput")
    out_d = nc.dram_tensor("out", [bl, c, n], BF16, kind="ExternalOutput")

    with tile.TileContext(nc) as tc:
        with (
            tc.tile_pool(name="const", bufs=1) as cpool,
            tc.tile_pool(name="xq", bufs=5) as xqp,
            tc.tile_pool(name="xt", bufs=4) as xtp,
            tc.tile_pool(name="act", bufs=5) as actp,
            tc.tile_pool(name="h", bufs=4) as hp,
            tc.tile_pool(name="outp", bufs=4) as outp,
            tc.tile_pool(name="psA", bufs=5, space="PSUM") as psA,
            tc.tile_pool(name="psW", bufs=3, space="PSUM") as psW,
        ):
            # --- constants.  Ring plan: item-0's xt8 chunks interleave
            # with at_sp quarters in consumption order on the two fast
            # HWDGE rings (they gate the first matmuls), then w+b on
            # scalar, then at_tm on sync behind those; later items' xt8 +
            # the early residuals ride gpsimd; out stores go on sync. ---
            at_sp = cpool.tile([P, kt, npr], F8, tag="at_sp")
            at_tm = cpool.tile([P, kt, npr], F8, tag="at_tm")
            w_sb = [cpool.tile([P, ct, c], F8, tag=f"w{i}", name=f"w{i}")
                    for i in range(3)]
            b_sb = [cpool.tile([P, ct], F32, tag=f"b{i}", name=f"b{i}")
                    for i in range(3)]
            id_sb = cpool.tile([P, P], BF16, tag="ident")
            warm = cpool.tile([P, 2, 512], F8, tag="warm")

            def emit_warmup():
                # ~3us of throwaway matmuls on a memset scratch: bridges
                # the initial DMA wait and un-throttles the PE HAM clock
                # gate (cold PE runs at 1.2 GHz for the first ~3.4us of
                # sustained work) so the first real matmuls start warm
                nc.vector.memset(warm[:, 0:2, 0:128], 0)
                wps = psA.tile([P, 512], F32, tag="psA", name="warm_ps")
                for i in range(26):
                    nc.tensor.matmul(
                        wps[:, :64], lhsT=warm[:, 0:2, 0:128],
                        rhs=warm[:, 0:2, 0:64],
                        start=(i == 0), stop=(i == 25), perf_mode=DR)
                return wps

            def emit_load_w1_b1():
                # w1/b1 gate the very first W-mm/A-mm drains: they must
                # precede the at_sp bulk on the scalar ring
                nc.scalar.dma_start(w_sb[0][:], w_d[0][:])
                nc.scalar.dma_start(b_sb[0][:], b_d[0][:])

            def emit_load_w_b():
                for i in (1, 2):
                    nc.scalar.dma_start(w_sb[i][:], w_d[i][:])
                    nc.scalar.dma_start(b_sb[i][:], b_d[i][:])
                nc.scalar.dma_start(id_sb[:], id_d[:])

            def emit_load_xt8(b):
                # fp8 transposed x (layer-1 W-mm stationary). Item 0 gates
                # the very first matmuls: its chunks interleave with at_sp
                # (pair, quarter) transfers in exactly the consumption
                # order of the first W-mm / A-mm, split across both fast
                # rings. Later items are 2 descriptors on gpsimd.
                # DMA efficiency wants few transfers with long contiguous
                # per-partition segments (the hw allows only ~4 in-flight
                # transfers per ring and small strided segments crawl):
                # one 1792B-line descriptor per cc. Items 0-1 gate the
                # first matmuls and ride the two fast rings in parallel;
                # later items ride gpsimd.
                # NOTE: the scalar queue carries no bulk DMA at all -- a
                # DMA issue there blocks behind the 4-deep in-flight
                # window and starves the Act-engine drains behind it
                xt8 = xtp.tile([P, ct, npad], F8, tag="xt8", name=f"xt8_{b}")
                if b <= 1:
                    # halves so the first W-mm chunks unblock at half the
                    # transfer; cc0 on sync, cc1 on gpsimd in parallel
                    nh = npad // 2
                    for cc, eng in ((0, nc.sync), (1, nc.gpsimd)):
                        eng.dma_start(xt8[:, cc, :nh],
                                      xt8_d[b, cc * P:(cc + 1) * P, :nh])
                        eng.dma_start(xt8[:, cc, nh:],
                                      xt8_d[b, cc * P:(cc + 1) * P, nh:])
                else:
                    for cc in range(ct):
                        nc.gpsimd.dma_start(xt8[:, cc, :],
                                            xt8_d[b, cc * P:(cc + 1) * P, :])
                return xt8

            def emit_load_at_sp():
                # whole (k-tile pair) descriptors -- 1712B contiguous
                # lines -- spread over sync/gpsimd in item-0 A-mm
                # consumption order, weighted toward gpsimd (measured:
                # the sync ring lags with its xt8 halves in front; the
                # gpsimd SWDGE moves big contiguous transfers fine)
                ring = {0: nc.sync, 1: nc.gpsimd, 2: nc.gpsimd, 3: nc.sync,
                        4: nc.gpsimd, 5: nc.sync, 6: nc.gpsimd}
                for k in range(kp):
                    ring[k].dma_start(at_sp[:, 2 * k:2 * k + 2, :],
                                      atsp_d[:, 2 * k:2 * k + 2, :])

            def emit_load_at_tm():
                # whole-pair descriptors in consumption order, sync ring
                # only: the scalar engine's queue must stay free for the
                # drains that start while at_tm is still loading
                for k in range(kp):
                    nc.sync.dma_start(at_tm[:, 2 * k:2 * k + 2, :],
                                      attm_d[:, 2 * k:2 * k + 2, :])

            def emit_load_xt(b):
                # bf16 transposed residual, needed only at the layer-3
                # drain: items 0-1 ride gpsimd behind the xt8 pairs, later
                # items the sync ring (which frees up after at_tm)
                eng = nc.gpsimd if b <= 1 else nc.sync
                xt = xtp.tile([P, ct, n], BF16, tag="xt", name=f"xt_{b}")
                for cc in range(ct):
                    eng.dma_start(xt[:, cc, :], xt_d[b, cc * P:(cc + 1) * P, :])
                return xt

            def emit_wmm(b, li, aT, name):
                # W-mm: h = a @ W in natural layout, from transposed a
                h = hp.tile([P, kt, c], F8, tag="h", name=f"{name}_{b}")
                for k in range(kt):
                    ps = psW.tile([P, 512], F32, tag="psW")
                    nc.tensor.matmul(
                        ps[:, :c],
                        lhsT=aT[:, 0:2, k * P:(k + 1) * P],
                        rhs=w_sb[li][:, 0:2, :],
                        start=True, stop=True, perf_mode=DR)
                    # alternate drains across DVE/Act so the copy chain
                    # keeps pace with the 256-col matmuls
                    if k % 2 == 0:
                        nc.vector.tensor_copy(h[:, k, :], ps[:, :c])
                    else:
                        nc.scalar.copy(h[:, k, :], ps[:, :c])
                return h

            def _amm_drain(aT, li, cc, ps, q0, qs):
                # drain a quarter as two parallel halves (scalar + DVE) so
                # the psA bank frees ~2x sooner; downstream matmuls wait
                # on these tiles
                hh = qs // 2
                nc.scalar.activation(aT[:, cc, q0:q0 + hh],
                                     ps[:, :hh], RELU,
                                     bias=b_sb[li][:, cc:cc + 1])
                nc.vector.tensor_scalar(
                    aT[:, cc, q0 + hh:q0 + qs], ps[:, hh:qs],
                    b_sb[li][:, cc:cc + 1], 0.0,
                    op0=mybir.AluOpType.add,
                    op1=mybir.AluOpType.max)

            def emit_amm(b, li, h, at, name):
                # A-mm: a'T = relu((A @ h)^T + bias), pair-outer per cc
                # over 4 parallel PSUM banks (consecutive matmuls MUST hit
                # different banks to pipeline -- same-bank back-to-back
                # accumulation stalls on the array drain, measured +55us).
                # Item 0/1 also consume each at pair as its DMA lands.
                aT = actp.tile([P, ct, npad], F8, tag="act",
                               name=f"{name}_{b}")
                for cc in range(ct):
                    # cols [n:npad] are read as the next W-mm's lhsT pads
                    # but never written by the trimmed quarters
                    nc.vector.memset(aT[:, cc, n:npad], 0)
                for cc in range(ct):
                    groups = [(psA.tile([P, 512], F32, tag="psA",
                                        name=f"ps{li}_{b}_{cc}_{q0}"),
                               q0, qs)
                              for (q0, qs) in nqv]
                    for k in range(kp):
                        for (ps, q0, qs) in groups:
                            nc.tensor.matmul(
                                ps[:, :qs],
                                lhsT=h[:, 2 * k:2 * k + 2,
                                       cc * P:(cc + 1) * P],
                                rhs=at[:, 2 * k:2 * k + 2, q0:q0 + qs],
                                start=(k == 0), stop=(k == kp - 1),
                                perf_mode=DR)
                        if b == 0 and li == 0 and cc == 0 and k < kp - 1:
                            # item-0's first A-mm paces on at_sp DMA
                            # arrivals: two throwaway matmuls per round
                            # keep the HAM clock gate warm through the
                            # stalls (reusing the warmup scratch/psum)
                            for _ in range(2):
                                nc.tensor.matmul(
                                    warm_ps[:, :64],
                                    lhsT=warm[:, 0:2, 0:128],
                                    rhs=warm[:, 0:2, 0:64],
                                    start=True, stop=True, perf_mode=DR)
                    for (ps, q0, qs) in groups:
                        _amm_drain(aT, li, cc, ps, q0, qs)
                return aT

            def emit_l1(b, xt8):
                h1 = emit_wmm(b, 0, xt8, "h1")
                return emit_amm(b, 0, h1, at_sp, "a1T")

            def emit_l3out(b, xt, h3, last):
                # layer-3 A-mm: outT = relu((A_sp @ h3)^T + xT + b3),
                # residual and bias in the drain. For the final item the
                # residual is accumulated on the PE via an identity matmul
                # so the drain (which nothing overlaps) is relu+store.
                for cc in range(ct):
                    groups = [(psA.tile([P, 512], F32, tag="psA",
                                        name=f"ps6_{b}_{cc}_{q0}"), q0, qs)
                              for (q0, qs) in nqv]
                    for k in range(kp):
                        for (ps, q0, qs) in groups:
                            nc.tensor.matmul(
                                ps[:, :qs],
                                lhsT=h3[:, 2 * k:2 * k + 2,
                                        cc * P:(cc + 1) * P],
                                rhs=at_sp[:, 2 * k:2 * k + 2, q0:q0 + qs],
                                start=(k == 0),
                                stop=(k == kp - 1 and not last),
                                perf_mode=DR)
                    if last:
                        for (ps, q0, qs) in groups:
                            nc.tensor.matmul(
                                ps[:, :qs], lhsT=id_sb[:, :],
                                rhs=xt[:, cc, q0:q0 + qs],
                                start=False, stop=True)
                    ot = outp.tile([P, n], BF16, tag="o")
                    for qi, (ps, q0, qs) in enumerate(groups):
                        if last:
                            # alternate Act/DVE so the unoverlapped final
                            # drain chain runs on two engines
                            if qi % 2 == 0:
                                nc.scalar.activation(
                                    ot[:, q0:q0 + qs], ps[:, :qs], RELU,
                                    bias=b_sb[2][:, cc:cc + 1])
                            else:
                                nc.vector.tensor_scalar(
                                    ot[:, q0:q0 + qs], ps[:, :qs],
                                    b_sb[2][:, cc:cc + 1], 0.0,
                                    op0=mybir.AluOpType.add,
                                    op1=mybir.AluOpType.max)
                        else:
                            nc.vector.tensor_add(ot[:, q0:q0 + qs],
                                                 ps[:, :qs],
                                                 xt[:, cc, q0:q0 + qs])
                            nc.scalar.activation(ot[:, q0:q0 + qs],
                                                 ot[:, q0:q0 + qs], RELU,
                                                 bias=b_sb[2][:, cc:cc + 1])
                    # one 3400B-line store per cc; the final item stores
                    # per quarter over three rings as drains complete,
                    # shortening the unoverlapped tail
                    if last:
                        for qi, (ps, q0, qs) in enumerate(groups):
                            eng = (nc.sync, nc.scalar, nc.gpsimd,
                                   nc.sync)[qi]
                            eng.dma_start(
                                out_d[b, cc * P:(cc + 1) * P, q0:q0 + qs],
                                ot[:, q0:q0 + qs])
                    else:
                        nc.sync.dma_start(out_d[b, cc * P:(cc + 1) * P, :],
                                          ot[:])

            # Emission order: item-0's xt8/at_sp interleave on the fast
            # rings; item-1's layer 1 is hoisted right behind item-0's so
            # the PE has matmul work queued before the first at_tm use
            # (its DMA trails at_sp). Each item's layer 1 is emitted two
            # items ahead, placed between W3(b) and the layer-3 A-mm so
            # its matmuls bridge the h3 drain latency.
            warm_ps = emit_warmup()
            xt8 = {0: emit_load_xt8(0)}
            emit_load_w1_b1()
            emit_load_at_sp()
            emit_load_w_b()
            if bl > 1:
                xt8[1] = emit_load_xt8(1)
            a1T = {0: emit_l1(0, xt8.pop(0))}
            xt = {0: emit_load_xt(0)}
            if bl > 1:
                xt[1] = emit_load_xt(1)
                emit_load_at_tm()
                a1T[1] = emit_l1(1, xt8.pop(1))
            else:
                emit_load_at_tm()
            h3_held = None
            for b in range(bl):
                h2 = emit_wmm(b, 1, a1T.pop(b), "h2")
                a2T = emit_amm(b, 1, h2, at_tm, "a2T")
                h3 = emit_wmm(b, 2, a2T, "h3")
                if b + 2 < bl:
                    xt8[b + 2] = emit_load_xt8(b + 2)
                    xt[b + 2] = emit_load_xt(b + 2)
                    a1T[b + 2] = emit_l1(b + 2, xt8.pop(b + 2))
                    emit_l3out(b, xt.pop(b), h3, last=False)
                elif b < bl - 1:
                    # no layer-1 filler left: hold this item's layer-3 so
                    # it bridges the last item's h3 drain latency instead
                    h3_held = h3
                else:
                    if h3_held is not None:
                        emit_l3out(b - 1, xt.pop(b - 1), h3_held,
                                   last=False)
                    emit_l3out(b, xt.pop(b), h3, last=True)

    nc.compile()
    return nc


def _norm_adj_T(edges, n, npad):
    """A^T padded to [npad, npad] in fp32. AT[m, j] = A[j, m] where
    out[j] += A[j, m] * h[m]; edge (r -> c) contributes dinv[r]*dinv[c] at
    AT[r, c]. Self loops included."""
    row = np.concatenate([edges[0], np.arange(n, dtype=np.int64)])
    col = np.concatenate([edges[1], np.arange(n, dtype=np.int64)])
    deg = np.bincount(col, minlength=n).astype(np.float32)
    dinv = np.zeros(n, np.float32)
    nz = deg > 0
    dinv[nz] = 1.0 / np.sqrt(deg[nz])
    norm = dinv[row] * dinv[col]
    at = np.zeros((npad, npad), np.float32)
    np.add.at(at, (row, col), norm)
    return at


def _tile_rows(a, kt):
    """[kt*P, F] -> [P, kt, F] so that [p, k, :] = a[k*P + p, :]."""
    return np.ascontiguousarray(
        a.reshape(kt, P, a.shape[-1]).transpose(1, 0, 2))


_PROGRAM_CACHE = {}


def _get_program(bl, n, c):
    key = (bl, n, c)
    if key not in _PROGRAM_CACHE:
        _PROGRAM_CACHE[key] = build_program(bl, n, c)
    return _PROGRAM_CACHE[key]


def run(inputs, trace=False, n_cores=N_CORES):
    x32 = np.asarray(inputs["x"], dtype=np.float32)
    xt = np.ascontiguousarray(x32.transpose(0, 2, 1)).astype(NP_BF16)
    npad_h = -(-(x32.shape[1] + 1) // P) * P
    xt8 = np.zeros((x32.shape[0], x32.shape[2], npad_h), NP_F8)
    xt8[:, :, :x32.shape[1]] = x32.transpose(0, 2, 1).astype(NP_F8)
    w1 = np.asarray(inputs["W1"], np.float32)
    w2 = np.asarray(inputs["W2"], np.float32)
    w3 = np.asarray(inputs["W3"], np.float32)
    b1 = np.asarray(inputs["b1"], np.float32)
    b2 = np.asarray(inputs["b2"], np.float32)
    b3 = np.asarray(inputs["b3"], np.float32)
    e_sp = np.asarray(inputs["keypoint_line_without_temporal"]).astype(np.int64)
    e_tm = np.asarray(inputs["keypoint_line_with_temporal"]).astype(np.int64)

    b_total, n, c = x32.shape
    bl = b_total // n_cores
    kt = -(-(n + 1) // P)
    npad = kt * P
    ct = c // P

    nc = _get_program(bl, n, c)

    npr = -(-n // 16) * 16
    at_sp = _tile_rows(
        _norm_adj_T(e_sp, n, npad)[:, :npr].astype(NP_F8), kt)
    at_tm = _tile_rows(
        _norm_adj_T(e_tm, n, npad)[:, :npr].astype(NP_F8), kt)
    shared = {
        "at_sp": at_sp,
        "at_tm": at_tm,
        "w1": _tile_rows(w1.astype(NP_F8), ct),
        "w2": _tile_rows(w2.astype(NP_F8), ct),
        "w3": _tile_rows(w3.astype(NP_F8), ct),
        "b1": np.ascontiguousarray(b1.reshape(ct, P).T),
        "b2": np.ascontiguousarray(b2.reshape(ct, P).T),
        "b3": np.ascontiguousarray(b3.reshape(ct, P).T),
        "ident": np.eye(P, dtype=NP_BF16),
    }
    in_maps = [
        {"xt": np.ascontiguousarray(xt[i * bl:(i + 1) * bl]),
         "xt8": np.ascontiguousarray(xt8[i * bl:(i + 1) * bl]), **shared}
        for i in range(n_cores)
    ]
    res = run_bass_kernel_spmd(nc, in_maps, core_ids=list(range(n_cores)),
                               trace=trace)
    out = np.concatenate(
        [np.asarray(r["out"]).astype(np.float32).transpose(0, 2, 1)
         for r in res.results], axis=0)
    return out, res


def kernel(**inputs) -> np.ndarray:
    out, _ = run(inputs, trace=False)
    return out
